# revision 41
# baseline (speedup 1.0000x reference)
"""Multi-head self-attention Trainium2 kernel (8 NeuronCores).

Sharding: 8 cores = 4 batches x 2 head-halves (6 heads each). Per core:
QKV projections run as fp8(e4m3) hi+lo DoubleRow matmuls (host splits X
and W into hi/lo fp8 pairs; the lo*lo term is dropped), scores S^T = K.Q^T
in fp16 per (head, key-tile, q-block), P = exp(S/8) on ScalarE straight
out of 3-bank PSUM groups, ctx in the natural [q, d] orientation
(lhsT = P^T slab slices, rhs = V with a ones column for the softmax
denominator) accumulated per (head, q-block) in a time-split PSUM bank,
normalization + PE transpose into ctx^T, and the out-projection
Y_partial = ctx @ Wo[:, cols]^T in fp16. The host sums the two per-batch
partials and adds the output bias.

Scheduling: each unit's ctx matmuls trail one unit behind the score/exp
stream (head-major combo order so the shared ctx PSUM bank is time-split
between the two heads); projections for the next head pair, V tiles,
transposes and out-projection slices drip into the rotation stream.
"""

import numpy as np

B, S, E, H, D = 4, 2048, 768, 12, 64
NCORES = 8

_cache = {}


def _emit(nc, tc, ctx, io):
    import concourse.mybir as mybir

    fp32 = mybir.dt.float32
    fp16 = mybir.dt.float16
    fp8 = mybir.dt.float8e4
    i16 = mybir.dt.int16
    Exp = mybir.ActivationFunctionType.Exp
    DR = mybir.MatmulPerfMode.DoubleRow

    Scfg, Ecfg, EL = 2048, 768, 384
    NKT = Ecfg // 128           # contraction tiles over embed dim (6)
    NS = Scfg // 128            # key tiles (16)
    NQB = Scfg // 512           # q-blocks (4)
    HP = EL // 128              # head pairs (3)
    ROT = 2                     # exp group size in PSUM banks
    NCOMBO = 2 * NS             # (head, k-tile) combos per unit (32)
    NROT = (NCOMBO + ROT - 1) // ROT  # 16
    # On offloaded rotations the LAST 512-col block of the exp slab moves
    # off the ScalarE: phase-A Schraudolph on DVE (int16-bitcast, reading
    # only that block's PSUM region so it overlaps the ScalarE exp of the
    # other blocks), phase B = bits-512 on DVE (cheap 4x int16 mode, = the
    # 2^-0.5-weighted half-period sample), then a single in-place add on
    # the otherwise idle GPSIMD. The (1+2^-0.5) normalization is folded
    # into the bias so the sum lands on full-scale exp — ~1.1% max element
    # error on the offloaded block's keys.
    SCH = frozenset(range(8, 184, 2))
    SCH_A = float(1024.0 * np.log2(np.e) / 8.0)
    SCH_B2 = float(1024.0 * (15.0 - 0.052) + 0.5
                   - 1024.0 * np.log2(1.0 + 2.0 ** -0.5))

    X8, W8Q, W8QS, W8K, W8KS, W8V, W8VS, WOT, BQ, BK, BV, IDT, Y = (
        io["X8"], io["W8Q"], io["W8QS"], io["W8K"], io["W8KS"], io["W8V"],
        io["W8VS"], io["WOT"], io["BQ"], io["BK"], io["BV"], io["IDT"],
        io["Y"],
    )

    consts = ctx.enter_context(tc.tile_pool(name="consts", bufs=1))
    wpool = ctx.enter_context(tc.tile_pool(name="wpool", bufs=1))
    xpool = ctx.enter_context(tc.tile_pool(name="xpool", bufs=1))
    qkpool = ctx.enter_context(tc.tile_pool(name="qkpool", bufs=4))
    vpool = ctx.enter_context(tc.tile_pool(name="vpool", bufs=1))
    spool = ctx.enter_context(tc.tile_pool(name="spool", bufs=33))
    eapool = ctx.enter_context(tc.tile_pool(name="eapool", bufs=3))
    cpool = ctx.enter_context(tc.tile_pool(name="cpool", bufs=1))
    npool = ctx.enter_context(tc.tile_pool(name="npool", bufs=4))
    ypool = ctx.enter_context(tc.tile_pool(name="ypool", bufs=6))
    psum_g = ctx.enter_context(tc.tile_pool(name="psum_g", bufs=3, space="PSUM"))
    psum_c = ctx.enter_context(tc.tile_pool(name="psum_c", bufs=1, space="PSUM"))
    psum_m = ctx.enter_context(tc.tile_pool(name="psum_m", bufs=1, space="PSUM"))

    # ---- weights / constants to SBUF (critical-path tensors first) ----
    # Chunk-major layouts: every DMA piece is one contiguous flat range, so
    # the (coarse, interval-based) dependency tracker never chains a matmul
    # to a transfer it does not actually read.
    #   X:   [p, chunk(4), h(2), k(6), 512 tok]
    #   Wq/Wk (+swaps): [p, colchunk(3), h(2), k(6), 128 cols]
    #   Wv (+swap):     [p, h(2), k(6), 384 cols]
    x8_sb = xpool.tile([128, NKT * 2 * Scfg], fp8, name="x8_sb")
    x5s = x8_sb.rearrange("p (c h k t) -> p c h k t", c=4, h=2, k=NKT)
    x5d = X8.rearrange("p (c h k t) -> p c h k t", c=4, h=2, k=NKT)

    def xchunk(tc_, h):
        nc.sync.dma_start(out=x5s[:, tc_, h], in_=x5d[:, tc_, h])

    wq_parts, wk_parts = [], []
    for nm, src, parts in (("w8q", W8Q, wq_parts), ("w8qs", W8QS, wq_parts),
                           ("w8k", W8K, wk_parts), ("w8ks", W8KS, wk_parts)):
        t = wpool.tile([128, NKT * 2 * EL], fp8, name=f"{nm}_sb")
        sv = t.rearrange("p (c h k w) -> p c h k w", c=HP, h=2, k=NKT)
        dv = src.rearrange("p (c h k w) -> p c h k w", c=HP, h=2, k=NKT)
        parts.append((sv, dv))
        if nm == "w8q":
            w5q = sv
        elif nm == "w8qs":
            w5qs = sv
        elif nm == "w8k":
            w5k = sv
        else:
            w5ks = sv

    # hi planes first, in first-use order: the first 3 DR matmuls only need
    # wq chunk0-hi and X chunk0-hi
    nc.sync.dma_start(out=wq_parts[0][0][:, 0, 0], in_=wq_parts[0][1][:, 0, 0])
    xchunk(0, 0)
    nc.sync.dma_start(out=wq_parts[0][0][:, 0, 1], in_=wq_parts[0][1][:, 0, 1])
    nc.sync.dma_start(out=wq_parts[1][0][:, 0, 0], in_=wq_parts[1][1][:, 0, 0])
    nc.sync.dma_start(out=wq_parts[1][0][:, 0, 1], in_=wq_parts[1][1][:, 0, 1])
    xchunk(0, 1)
    bq_sb = consts.tile([128, HP], fp32, name="bq_sb")
    nc.sync.dma_start(out=bq_sb, in_=BQ)
    bk_sb = consts.tile([128, HP], fp32, name="bk_sb")
    nc.sync.dma_start(out=bk_sb, in_=BK)
    for sv, dv in wk_parts:
        nc.sync.dma_start(out=sv[:, 0, 0], in_=dv[:, 0, 0])
        nc.sync.dma_start(out=sv[:, 0, 1], in_=dv[:, 0, 1])
    xchunk(1, 0)
    xchunk(1, 1)
    bv_sb = consts.tile([128, EL], fp32, name="bv_sb")
    nc.sync.dma_start(out=bv_sb, in_=BV)
    idt_sb = consts.tile([128, 128], fp16, name="idt_sb")
    nc.sync.dma_start(out=idt_sb, in_=IDT)
    w8v_sb = wpool.tile([128, NKT * 2 * EL], fp8, name="w8v_sb")
    w5v = w8v_sb.rearrange("p (h k c) -> p h k c", h=2, k=NKT)
    w8vs_sb = wpool.tile([128, NKT * 2 * EL], fp8, name="w8vs_sb")
    w5vs = w8vs_sb.rearrange("p (h k c) -> p h k c", h=2, k=NKT)
    for t_, src in ((w8v_sb, W8V), (w8vs_sb, W8VS)):
        sv = t_.rearrange("p (h x) -> p h x", h=2)
        dv = src.rearrange("p (h x) -> p h x", h=2)
        nc.sync.dma_start(out=sv[:, 0], in_=dv[:, 0])
        nc.sync.dma_start(out=sv[:, 1], in_=dv[:, 1])
    for tc_ in (2, 3):
        xchunk(tc_, 0)
        xchunk(tc_, 1)

    # late-needed weights (hp1/hp2 Q,K column chunks; Wo) issue from dripped
    # closures instead of jamming the startup DMA queue
    wo_sb = []
    for hp in range(HP):
        t = wpool.tile([128, Ecfg], fp16, name=f"wo{hp}_sb", tag=f"wo{hp}")
        wo_sb.append(t)

    def build_wdma():
        closures = []
        for ch in (1, 2):
            for parts in (wq_parts, wk_parts):
                def cl(ch=ch, parts=parts):
                    for sv, dv in parts:
                        nc.sync.dma_start(out=sv[:, ch, 0], in_=dv[:, ch, 0])
                        nc.sync.dma_start(out=sv[:, ch, 1], in_=dv[:, ch, 1])
                closures.append(cl)

        def cl_wo():
            for hp in range(HP):
                nc.sync.dma_start(out=wo_sb[hp],
                                  in_=WOT[hp * 128:(hp + 1) * 128, :])
        closures.append(cl_wo)
        return closures

    x5 = x5s

    ctxT = []
    for hp in range(HP):
        t = cpool.tile([128, Scfg], fp16, name=f"ctxT{hp}", tag=f"ctx{hp}")
        ctxT.append(t)

    import collections
    pending = collections.deque()
    chain = [None]   # a closure that MUST be the next pop (its PSUM bank is
                     # mid-accumulation — nothing may allocate in between)

    def drip(n=1):
        for _ in range(n):
            if chain[0] is not None:
                cl, chain[0] = chain[0], None
            elif pending:
                cl = pending.popleft()
            else:
                return
            cl()

    # ---- fp8 hi/lo DoubleRow projection closures ----
    def build_proj(hp):
        qt = qkpool.tile([128, Scfg], fp16, name=f"qt{hp}", tag="qt")
        kt = qkpool.tile([128, Scfg], fp16, name=f"kt{hp}", tag="kt")
        hsl = slice(hp * 128, (hp + 1) * 128)
        closures = []
        for dst, w5, w5s, bsb in ((qt, w5q, w5qs, bq_sb), (kt, w5k, w5ks, bk_sb)):
            for nb in range(NQB):
                def cl(dst=dst, w5=w5, w5s=w5s, bsb=bsb, nb=nb, hp=hp):
                    tb = slice(nb * 512, (nb + 1) * 512)
                    ps = psum_m.tile([128, 512], fp32, name=f"pj{hp}_{nb}",
                                     tag="m")
                    for j, k in enumerate(range(0, NKT, 2)):
                        nc.tensor.matmul(
                            ps, lhsT=w5[:, hp, 0, k:k + 2, :],
                            rhs=x5[:, nb, 0, k:k + 2, :],
                            start=(j == 0), stop=False, perf_mode=DR)

                    def cl2(ps=ps):
                        for k in range(NKT):
                            nc.tensor.matmul(
                                ps, lhsT=w5s[:, hp, :, k, :],
                                rhs=x5[:, nb, :, k, :],
                                start=False, stop=(k == NKT - 1), perf_mode=DR)
                        nc.vector.tensor_scalar(
                            dst[:, tb], ps, 1.0 / 64.0, bsb[:, hp:hp + 1],
                            mybir.AluOpType.mult, mybir.AluOpType.add)
                    chain[0] = cl2
                closures.append(cl)
        return qt, kt, closures

    # ---- V for ALL heads: per key-tile [128 tok, 6*65] fp16, ones col per
    # head for the softmax denominator ----
    v_sb = [vpool.tile([128, 65 * 2 * HP], fp16, name=f"v_{si}", tag=f"v{si}")
            for si in range(NS)]

    v_done = [False] * NS

    def build_v():
        closures = []
        for si in range(NS):
            def cl(si=si, vt=v_sb[si]):
                c, off = divmod(si, 4)
                osl = slice(off * 128, off * 128 + 128)
                nc.gpsimd.memset(vt, 1.0)
                ps = psum_m.tile([128, EL], fp32, name=f"pv{si}", tag="m")
                for j, k in enumerate(range(0, NKT, 2)):
                    nc.tensor.matmul(
                        ps, lhsT=x5[:, c, 0, k:k + 2, osl],
                        rhs=w5v[:, 0, k:k + 2, :],
                        start=(j == 0), stop=False, perf_mode=DR)

                def cl2(si=si, vt=vt, ps=ps, c=c, osl=osl):
                    for k in range(NKT):
                        nc.tensor.matmul(
                            ps, lhsT=x5[:, c, :, k, osl],
                            rhs=w5vs[:, :, k, :],
                            start=False, stop=(k == NKT - 1), perf_mode=DR)
                    nc.vector.scalar_tensor_tensor(
                        vt.rearrange("p (h w) -> p h w", w=65)[:, :, 0:64],
                        ps.rearrange("p (h w) -> p h w", w=64),
                        1.0 / 64.0,
                        bv_sb.rearrange("p (h w) -> p h w", w=64),
                        mybir.AluOpType.mult, mybir.AluOpType.add)
                    v_done[si] = True
                chain[0] = cl2
            closures.append(cl)
        return closures

    # ---- out-projection: Y[si] = ctx @ Wo_loc^T (fp16), fp16 DMA out ----
    def build_y(si, gp=False):
        # two closures (one per 384-col chunk) so the drip stays fine-grained
        cell = {}

        def chunk(nh, si=si):
            if nh == 0:
                cell["y"] = ypool.tile([128, Ecfg], fp16, name=f"y{si}",
                                       tag="y")
            y_sb = cell["y"]
            # tail closures fan out over the ctx bank and a freed score
            # bank (idle after the last exp) to deepen the drain pipeline
            if gp:
                if nh == 1:
                    p = psum_g if si % 2 else psum_c
                else:
                    p = psum_g if si % 2 == 0 else psum_m
            else:
                p = psum_m
            tagmap = {id(psum_c): "c", id(psum_g): "g", id(psum_m): "m"}
            yps = p.tile([128, 384], fp32, name=f"yp{si}_{nh}",
                         tag=tagmap[id(p)])
            for hp in range(HP):
                nc.tensor.matmul(
                    yps, lhsT=ctxT[hp][:, si * 128:(si + 1) * 128],
                    rhs=wo_sb[hp][:, nh * 384:(nh + 1) * 384],
                    start=(hp == 0), stop=(hp == HP - 1))
            if gp and nh == 0:
                # tail: ACT is idle — split the drains across engines
                nc.scalar.copy(y_sb[:, nh * 384:(nh + 1) * 384], yps)
            else:
                nc.vector.tensor_copy(y_sb[:, nh * 384:(nh + 1) * 384], yps)
            if nh == 1:
                nc.sync.dma_start(out=Y[si * 128:(si + 1) * 128, :], in_=y_sb)
        return [lambda nh=nh: chunk(nh) for nh in range(2)]

    y_by_qb = {qb: [cl for si in range(qb * (NS // NQB),
                                       (qb + 1) * (NS // NQB))
                    for cl in build_y(si, gp=(qb == NQB - 1))]
               for qb in range(NQB)}

    # ---- trailing ctx: natural [q, 65] orientation, accumulated per
    # (head, q-block) in a time-split PSUM bank; on the head's last k-tile,
    # normalize (DVE) and queue the PE transpose into the drip stream so
    # the PE never waits on the normalize chain ----
    def build_transp(hp_u, qb_u, hh, ctxn):
        def cl():
            # the last unit's transposes run after the final exp — use a
            # freed score bank to keep the misc bank clear for y
            last = (hp_u == HP - 1 and qb_u == NQB - 1)
            tp = (psum_g if last else psum_m).tile(
                [64, 512], fp16, name=f"tp{hp_u}_{qb_u}_{hh}",
                tag="g" if last else "m")
            for qt in range(4):
                nc.tensor.matmul(
                    tp[:, qt * 128:qt * 128 + 128],
                    lhsT=ctxn[:, qt * 64:qt * 64 + 64],
                    rhs=idt_sb, is_transpose=True)
            nc.vector.tensor_copy(
                ctxT[hp_u][hh * 64:hh * 64 + 64,
                           qb_u * 512:qb_u * 512 + 512], tp)
            if hp_u == HP - 1 and hh == 1:
                # ctxT for this q-block is complete on every head pair —
                # release its out-projection closures (ordering-safe for any
                # trail lag)
                pending.extend(y_by_qb[qb_u])
        return cl

    # PSUM start=True lazily zeroes the whole 2KB bank, so the four 65-col
    # ctx regions sharing a bank must accumulate strictly one-after-another
    # (qt-major): a region's 16-step accumulation may not interleave with a
    # sibling region's start.
    def ctx_item(u, hh, qt, k):
        h6 = 2 * u["hp"] + hh
        if qt == 0 and k == 0:
            u["cps"] = psum_c.tile([128, 4 * 65], fp32,
                                   name=f"cps{u['hp']}_{u['qb']}_{hh}",
                                   tag="c")
        cps = u["cps"]
        r, j = divmod(hh * NS + k, ROT)
        slab = u["slabs"][r]
        csl = slice(qt * 65, qt * 65 + 65)
        ssl = slice(j * 512 + qt * 128, j * 512 + qt * 128 + 128)
        nc.tensor.matmul(
            cps[:, csl], lhsT=slab[:, ssl],
            rhs=v_sb[k][:, h6 * 65:h6 * 65 + 65],
            start=(k == 0), stop=(k == NS - 1))
        if qt == 3 and k == NS - 1:
            hp_u, qb_u = u["hp"], u["qb"]
            c3 = cps.rearrange("p (qt c) -> p qt c", c=65)
            rc = npool.tile([128, 4], fp32, name=f"rc{hp_u}_{qb_u}_{hh}",
                            tag="rc")
            nc.vector.reciprocal(rc, c3[:, :, 64:65])
            ctxn = npool.tile([128, 256], fp16, name=f"cn{hp_u}_{qb_u}_{hh}",
                              tag="cn")
            # single fused per-qt normalize via a 0-stride broadcast of rc
            nc.vector.tensor_mul(
                ctxn.rearrange("p (q c) -> p q c", c=64),
                c3[:, :, 0:64],
                rc.unsqueeze(-1).broadcast_to([128, 4, 64]))
            # delay the PE transpose ~2 rotations so it never heads the PE
            # queue while this normalize chain is still in flight
            transp_q.append((gi_now[0] + 2, build_transp(hp_u, qb_u, hh, ctxn)))

    # global rotation history for the lag-3 ctx trail; trail items are
    # single matmuls (unit, hh, qt, k) gated on their slab's exp rotation
    LAG = 27
    hist = []            # cumulative combos available after each global rot
    trail_q = collections.deque()   # (unit, hh, qt, k, gate)
    transp_q = collections.deque()  # (due_rotation, transpose closure)
    gi_now = [0]                    # current global rotation (for ctx_item)

    def trail_to(target, cap=48):
        # pause before a fresh PSUM bank (hh, qt=0, k=0) so the previous
        # half-unit's normalize has a rotation of shadow
        emitted = 0
        while trail_q and emitted < cap:
            u, hh, qt, k, gate = trail_q[0]
            if gate >= target or not v_done[k]:
                break
            if emitted and qt == 0 and k == 0:
                break
            trail_q.popleft()
            ctx_item(u, hh, qt, k)
            emitted += 1

    cur = build_proj(0)
    vcl = build_v()
    qt_dbg = None
    for hp in range(HP):
        qt, kt, closures = cur
        if hp == 0:
            qt_dbg = (qt, kt)
        if hp == 0:
            # minimal prefix so unit (0,0) can start; keys first, V paced
            # to arrive before the (deferred) ctx trail consumes it.
            closures[0]()
            if chain[0] is not None:
                c2, chain[0] = chain[0], None
                c2()
            # k0 inline in two token-halves on separate PSUM banks (the ctx
            # bank is idle this early) so the first score keys drain sooner
            for half, hpool in ((0, psum_m), (1, psum_c)):
                tb = slice(half * 256, half * 256 + 256)
                ps = hpool.tile([128, 256], fp32, name=f"pk0_{half}",
                                tag="m" if hpool is psum_m else "c")
                for j, k in enumerate(range(0, NKT, 2)):
                    nc.tensor.matmul(
                        ps, lhsT=w5k[:, 0, 0, k:k + 2, :],
                        rhs=x5[:, 0, 0, k:k + 2, tb],
                        start=(j == 0), stop=False, perf_mode=DR)
                for k in range(NKT):
                    nc.tensor.matmul(
                        ps, lhsT=w5ks[:, 0, :, k, :],
                        rhs=x5[:, 0, :, k, tb],
                        start=False, stop=(k == NKT - 1), perf_mode=DR)
                nc.vector.tensor_scalar(
                    kt[:, tb], ps, 1.0 / 64.0, bk_sb[:, 0:1],
                    mybir.AluOpType.mult, mybir.AluOpType.add)
            krest = closures[NQB + 1:2 * NQB]
            qrest = closures[1:NQB]
            pend0 = (krest[:2] + qrest[:1] + krest[2:] + qrest[1:]
                     + build_wdma() + vcl)
            pending.extend(pend0)
        cur = build_proj(hp + 1) if hp + 1 < HP else None
        if cur is not None:
            pending.extend(cur[2])

        if hp == 1 and "DQT" in io:
            nc.sync.dma_start(out=io["DQT"], in_=qt_dbg[0])
            nc.sync.dma_start(out=io["DKT"], in_=qt_dbg[1])
            nc.sync.dma_start(out=io["DV0"], in_=v_sb[0])
        for qb in range(NQB):
            qsl = slice(qb * 512, qb * 512 + 512)
            unit = dict(hp=hp, qb=qb, slabs=[], cps=None)
            first_unit = (hp == 0 and qb == 0)
            base = NCOMBO * (hp * NQB + qb)
            for hh_ in range(2):
                for qt_ in range(4):
                    for k_ in range(NS):
                        gate = base + hh_ * NS + (k_ if qt_ == 0 else NS - 1)
                        trail_q.append((unit, hh_, qt_, k_, gate))
            for r in range(NROT):
                cis = range(ROT * r, min(ROT * r + ROT, NCOMBO))
                n = len(cis)
                g = psum_g.tile([128, 512 * ROT], fp32,
                                name=f"g{hp}_{qb}_{r}", tag="g")
                for j, ci in enumerate(cis):
                    hh, k = divmod(ci, NS)
                    nc.tensor.matmul(
                        g[:, j * 512:(j + 1) * 512],
                        lhsT=kt[hh * 64:(hh + 1) * 64, k * 128:(k + 1) * 128],
                        rhs=qt[hh * 64:(hh + 1) * 64, qsl],
                        start=True, stop=True)
                slab = spool.tile([128, 512 * ROT], fp16,
                                  name=f"s{hp}_{qb}_{r}", tag="slab")
                # trail first: the normalize chain it may emit must sit
                # ahead of the (long) offloaded exp ops in the DVE queue,
                # and the score-bank recycle has two groups of slack
                hist.append((hist[-1] if hist else 0) + n)
                gi_cur = len(hist) - 1
                gi_now[0] = gi_cur
                if gi_cur >= 2 * LAG:
                    trail_to(hist[gi_cur - LAG])
                while transp_q and transp_q[0][0] <= gi_cur:
                    transp_q.popleft()[1]()
                if gi_cur in SCH and n >= 2:
                    bsl = slice((n - 1) * 512, n * 512)
                    ea = eapool.tile([128, 512], fp16,
                                     name=f"ea{hp}_{qb}_{r}", tag="ea")
                    # phase A (DVE, reads only the last block's PSUM region)
                    nc.vector.tensor_scalar(
                        ea.bitcast(i16), g[:, bsl],
                        SCH_A, SCH_B2, mybir.AluOpType.mult,
                        mybir.AluOpType.add)
                    # phase B bits = bits(A) - 512 (DVE 4x int16)
                    nc.vector.tensor_scalar(
                        slab[:, bsl].bitcast(i16), ea.bitcast(i16),
                        -512, None, mybir.AluOpType.add)
                    # average on GPSIMD: slab block += ea (in place)
                    nc.gpsimd.tensor_add(slab[:, bsl], slab[:, bsl], ea)
                    # ScalarE keeps the leading blocks
                    nc.scalar.activation(slab[:, :(n - 1) * 512],
                                         g[:, :(n - 1) * 512],
                                         Exp, scale=0.125)
                    unit["slabs"].append(slab)
                    drip(1)
                elif first_unit and r == 0:
                    # per-combo exps so ScalarE starts right after the very
                    # first score matmul instead of after the whole rotation
                    for j in range(n):
                        jsl = slice(j * 512, (j + 1) * 512)
                        nc.scalar.activation(slab[:, jsl], g[:, jsl],
                                             Exp, scale=0.125)
                    unit["slabs"].append(slab)
                    drip(1)
                else:
                    nc.scalar.activation(slab[:, :n * 512], g[:, :n * 512],
                                         Exp, scale=0.125)
                    unit["slabs"].append(slab)
                    drip(1)
                drip(2 if first_unit else 0)

    # drain the remaining ctx matmuls, dripping in the leftover closures
    nt = 0
    while trail_q:
        u, hh, qt, k, gate = trail_q.popleft()
        ctx_item(u, hh, qt, k)
        while transp_q and len(trail_q) < 64:
            transp_q.popleft()[1]()
        nt += 1
        if nt % 3 == 0:
            drip(1)
    while transp_q:
        transp_q.popleft()[1]()
    drip(10000)
    if "DCT" in io:
        for hp in range(HP):
            nc.sync.dma_start(
                out=io["DCT"][hp * 128:(hp + 1) * 128, :], in_=ctxT[hp])


def _build():
    import contextlib
    import concourse.mybir as mybir
    import concourse.tile as tile
    from concourse import bacc

    fp32, fp16 = mybir.dt.float32, mybir.dt.float16
    fp8 = mybir.dt.float8e4
    Scfg, Ecfg, EL, NKT, HP = 2048, 768, 384, 6, 3

    nc = bacc.Bacc("TRN2", target_bir_lowering=False, debug=False,
                   num_devices=NCORES)
    io = {
        "X8": nc.dram_tensor("X8", [128, NKT * 2 * Scfg], fp8,
                             kind="ExternalInput").ap(),
        "W8Q": nc.dram_tensor("W8Q", [128, NKT * 2 * EL], fp8,
                              kind="ExternalInput").ap(),
        "W8QS": nc.dram_tensor("W8QS", [128, NKT * 2 * EL], fp8,
                               kind="ExternalInput").ap(),
        "W8K": nc.dram_tensor("W8K", [128, NKT * 2 * EL], fp8,
                              kind="ExternalInput").ap(),
        "W8KS": nc.dram_tensor("W8KS", [128, NKT * 2 * EL], fp8,
                               kind="ExternalInput").ap(),
        "W8V": nc.dram_tensor("W8V", [128, NKT * 2 * EL], fp8,
                              kind="ExternalInput").ap(),
        "W8VS": nc.dram_tensor("W8VS", [128, NKT * 2 * EL], fp8,
                               kind="ExternalInput").ap(),
        "WOT": nc.dram_tensor("WOT", [EL, Ecfg], fp16,
                              kind="ExternalInput").ap(),
        "BQ": nc.dram_tensor("BQ", [128, HP], fp32, kind="ExternalInput").ap(),
        "BK": nc.dram_tensor("BK", [128, HP], fp32, kind="ExternalInput").ap(),
        "BV": nc.dram_tensor("BV", [128, EL], fp32, kind="ExternalInput").ap(),
        "IDT": nc.dram_tensor("IDT", [128, 128], fp16,
                              kind="ExternalInput").ap(),
        "Y": nc.dram_tensor("Y", [Scfg, Ecfg], fp16,
                            kind="ExternalOutput").ap(),
    }
    with tile.TileContext(nc) as tc:
        with contextlib.ExitStack() as ctx:
            _emit(nc, tc, ctx, io)
    nc.compile()
    return nc


def _get_program():
    if "full" not in _cache:
        _cache["full"] = _build()
    return _cache["full"]


def _hilo(a):
    """fp32 array -> (hi, lo) e4m3 pair."""
    import ml_dtypes
    e4 = ml_dtypes.float8_e4m3
    hi = a.astype(e4)
    lo = (a - hi.astype(np.float32)).astype(e4)
    return hi, lo


def _k_major(a, ncols):
    """[rows=k*128, ncols] -> [128, k, ncols] (contraction-tile-major)."""
    k = a.shape[0] // 128
    return np.ascontiguousarray(a.reshape(k, 128, ncols).transpose(1, 0, 2))


def _half_inputs(half, Wq, bq, Wk, bk, Wv, bv, Wo):
    f16 = np.float16
    e0 = 384 * half
    ecols = slice(e0, e0 + 384)
    out = {}
    for nm, W in (("Q", Wq), ("K", Wk)):
        # x64 pre-scale keeps the lo residual above e4m3's subnormal floor;
        # the PSUM drain multiplies by 1/64.
        wt = _k_major(np.ascontiguousarray(W[ecols, :].T) * 64.0, 384)
        hi, lo = _hilo(wt)  # [128, 6, 384]
        # [p, colchunk(3), h(2), k(6), 128]
        norm = np.stack([hi, lo], axis=1).reshape(128, 2, 6, 3, 128)
        norm = norm.transpose(0, 3, 1, 2, 4)
        swap = np.stack([lo, hi], axis=1).reshape(128, 2, 6, 3, 128)
        swap = swap.transpose(0, 3, 1, 2, 4)
        out[f"W8{nm}"] = np.ascontiguousarray(norm).reshape(128, -1)
        out[f"W8{nm}S"] = np.ascontiguousarray(swap).reshape(128, -1)
    wt = _k_major(np.ascontiguousarray(Wv[ecols, :].T) * 64.0, 384)
    hi, lo = _hilo(wt)  # [p, h(2), k(6), 384]
    out["W8V"] = np.ascontiguousarray(
        np.stack([hi, lo], axis=1)).reshape(128, -1)
    out["W8VS"] = np.ascontiguousarray(
        np.stack([lo, hi], axis=1)).reshape(128, -1)
    out["WOT"] = np.ascontiguousarray(Wo[:, ecols].T).astype(f16)
    out["BQ"] = np.ascontiguousarray(bq[ecols].reshape(3, 128).T).astype(np.float32)
    out["BK"] = np.ascontiguousarray(bk[ecols].reshape(3, 128).T).astype(np.float32)
    out["BV"] = np.ascontiguousarray(
        np.broadcast_to(bv[ecols], (128, 384))).astype(np.float32)
    out["IDT"] = np.eye(128, dtype=f16)
    return out


def kernel(X, Wq, bq, Wk, bk, Wv, bv, Wo, bo):
    from concourse.bass_utils import run_bass_kernel_spmd

    X, Wq, bq, Wk, bk, Wv, bv, Wo, bo = [
        np.asarray(a, dtype=np.float32)
        for a in (X, Wq, bq, Wk, bk, Wv, bv, Wo, bo)
    ]
    nc = _get_program()
    halves = [_half_inputs(h, Wq, bq, Wk, bk, Wv, bv, Wo) for h in range(2)]
    x8s = []
    for b in range(B):
        xt = _k_major(np.ascontiguousarray(X[b].T), 2048)
        hi, lo = _hilo(xt)  # [128, 6, 2048]
        # [p, chunk(4), h(2), k(6), 512]
        x = np.stack([hi, lo], axis=1).reshape(128, 2, 6, 4, 512)
        x8s.append(np.ascontiguousarray(x.transpose(0, 3, 1, 2, 4)).reshape(128, -1))
    in_maps = [dict(halves[c % 2], X8=x8s[c // 2]) for c in range(NCORES)]
    res = run_bass_kernel_spmd(nc, in_maps, list(range(NCORES)))
    out = np.empty((B, S, E), np.float32)
    for b in range(B):
        out[b] = (res.results[2 * b]["Y"].astype(np.float32)
                  + res.results[2 * b + 1]["Y"].astype(np.float32)
                  + bo[None, :])
    return out

